# revision 17
# baseline (speedup 1.0000x reference)
"""BandPass biquad (torchaudio bandpass_biquad, const_skirt_gain=False) on 8 Trainium2 cores.

Strategy
--------
The biquad is an order-2 IIR with complex poles at radius ~0.691. Its impulse
response decays below 1e-10 after ~64 taps, so in fp32 the filter is *exactly*
(to fp32 precision) a 64-tap causal FIR:  y = conv(x, g),  g = conv([b0,0,b2], h),
h = impulse response of 1/(1 + a1 z^-1 + a2 z^-2).

The convolution maps onto the TensorEngine as banded-Toeplitz matmuls:
  out[q, n] = y[t0 + 128 n + q]
            = sum_p WA[p, q] x[t0 + 128 n + p]  +  sum_p WB[p, q] x[t0 + 128 (n-1) + p]
with WA[p, q] = g[q - p], WB[p, q] = g[q + 128 - p] (zero outside 0 <= . < K).
Two accumulating matmuls per output tile; no sequential recurrence anywhere.

Data is staged host-side into a time-across-partitions layout
x_T[p, c] = x[128 c + p] (one zero pad column per sequence for the t<0 state),
so the device does only full-burst natural DMAs, matmuls, and PSUM->SBUF copies.

Sharding: pure data parallel, 8 sequences per core (batch 64 over 8 cores).

Modes (BANDPASS_MODE env var):
  fp32  - 2 fp32 matmuls / tile.                     rel err ~2e-7
  bf16  - x and g split into bf16 hi+lo parts; 3 group pairs of full-rate bf16
          matmuls (x_hi*g1, x_hi*g2, x_lo*g1) accumulated in fp32 PSUM.
          rel err ~4e-6, ~4x less PE time than fp32.
"""

import math
import os

import ml_dtypes
import numpy as np

# ---------------------------------------------------------------- constants
SR = 48000.0
CENTRAL_FREQ = 4000.0
Q = 0.707

_w0 = 2.0 * math.pi * CENTRAL_FREQ / SR
_alpha = math.sin(_w0) / (2.0 * Q)
_a0 = 1.0 + _alpha
B0 = _alpha / _a0
B2 = -_alpha / _a0
A1 = (-2.0 * math.cos(_w0)) / _a0
A2 = (1.0 - _alpha) / _a0

BATCH, T = 64, 480000
N_CORES = 8
SEQ_PER_CORE = BATCH // N_CORES      # 8
P = 128                              # partitions / conv window
C = T // P                           # 3750 columns per sequence
NT = 375                             # matmul free-dim tile
NTILES = C // NT                     # 10
GROUP = 5                            # tiles per weight-pass group
K = 64                               # FIR taps (|g[63]| ~ 1e-10)
CP = C + 1                           # padded cols per sequence (leading zero col)

MODE = os.environ.get("BANDPASS_MODE", "bf16")


def _taps() -> np.ndarray:
    h = np.zeros(K + 2)
    h[0] = 1.0
    h[1] = -A1
    for n in range(2, K + 2):
        h[n] = -A1 * h[n - 1] - A2 * h[n - 2]
    g = B0 * h
    g[2:] += B2 * h[:-2]
    return g[:K]


def _weights() -> tuple[np.ndarray, np.ndarray]:
    g = _taps()
    p = np.arange(P)[:, None]
    q = np.arange(P)[None, :]
    ka = q - p
    wa = np.where((ka >= 0) & (ka < K), g[np.clip(ka, 0, K - 1)], 0.0)
    kb = q + P - p
    wb = np.where((kb >= 0) & (kb < K), g[np.clip(kb, 0, K - 1)], 0.0)
    return wa.astype(np.float32), wb.astype(np.float32)


# ---------------------------------------------------------------- device kernel
_BUILT = {}


def _build(mode: str):
    """Build the per-core Bass program (same NEFF for all 8 cores)."""
    import concourse.bacc as bacc
    import concourse.mybir as mybir
    import concourse.tile as tile

    nc = bacc.Bacc("TRN2", target_bir_lowering=False, debug=False)

    cp_total = SEQ_PER_CORE * CP
    co_total = SEQ_PER_CORE * C
    f32 = mybir.dt.float32
    bf16 = mybir.dt.bfloat16

    if mode == "fp32":
        x_names = ["xt"]
        w_names = ["wa", "wb"]
        xdt = f32
        # (weight name, x tensor idx, shifted)   shifted=False -> A chunk
        passes = [("wa", 0, False), ("wb", 0, True)]
    else:
        x_names = ["xh", "xl"]
        w_names = ["wa1", "wb1", "wa2", "wb2"]
        xdt = bf16
        passes = [
            ("wa1", 0, False), ("wb1", 0, True),
            ("wa2", 0, False), ("wb2", 0, True),
            ("wa1", 1, False), ("wb1", 1, True),
        ]

    x_aps = [
        nc.dram_tensor(nm, [P, cp_total], xdt, kind="ExternalInput").ap()
        for nm in x_names
    ]
    w_aps = {
        nm: nc.dram_tensor(nm, [P, P], xdt, kind="ExternalInput").ap()
        for nm in w_names
    }
    y_ap = nc.dram_tensor("yt", [P, co_total], f32, kind="ExternalOutput").ap()

    GC = GROUP * NT                  # cols per chunk (1875)
    with tile.TileContext(nc) as tc:
        with (
            tc.tile_pool(name="wpool", bufs=1) as wpool,
            tc.tile_pool(name="xpool", bufs=4) as xpool,
            tc.tile_pool(name="ypool", bufs=4) as ypool,
            tc.tile_pool(name="psum", bufs=8, space="PSUM") as psum_pool,
        ):
            # tiny weight loads first, on the ACT HWDGE ring so they don't
            # delay the first x chunk on the SP ring
            w_tiles = {}
            for nm in w_names:
                wt = wpool.tile([P, P], xdt, tag=nm, name=f"w_{nm}")
                nc.scalar.dma_start(wt[:], w_aps[nm][:])
                w_tiles[nm] = wt

            def groups_for_seq(s):
                # small leading chunks on the first sequence so the PE and the
                # output stream start early; small trailing chunks on the last
                # sequence so the post-last-input tail (matmul+copy+out-DMA)
                # is short. 10 tiles per sequence total.
                if s == 0:
                    return [(0, 1), (1, 2), (3, 2), (5, 5)]
                if s == SEQ_PER_CORE - 1:
                    return [(0, 5), (5, 3), (8, 2)]
                return [(0, GROUP), (GROUP, NTILES - GROUP)]

            for s in range(SEQ_PER_CORE):
                for g0, gn in groups_for_seq(s):
                    gc = gn * NT
                    # x chunk: gc data cols + 1 leading col (zero pad / overlap),
                    # padded-slab cols [s*CP + g0*NT, ... + gc + 1)
                    xs = []
                    for xi, _nm in enumerate(x_names):
                        xc = xpool.tile([P, gc + 1], xdt, tag=f"x{xi}",
                                        name=f"x{xi}_{s}_{g0}")
                        b = s * CP + g0 * NT
                        nc.sync.dma_start(xc[:], x_aps[xi][:, b:b + gc + 1])
                        xs.append(xc)

                    yg = ypool.tile([P, gc], f32, tag="y", name=f"y_{s}_{g0}")
                    ptiles = [
                        psum_pool.tile([P, NT], f32, tag="ps", name=f"ps{t}")
                        for t in range(gn)
                    ]
                    for pi, (wnm, xi, shifted) in enumerate(passes):
                        for t in range(gn):
                            c0 = t * NT + (0 if shifted else 1)
                            nc.tensor.matmul(
                                ptiles[t][:],
                                w_tiles[wnm][:],
                                xs[xi][:, c0:c0 + NT],
                                start=(pi == 0),
                                stop=(pi == len(passes) - 1),
                            )
                    for t in range(gn):
                        nc.vector.tensor_copy(yg[:, t * NT:(t + 1) * NT],
                                              ptiles[t][:])

                    # output chunks on the ACT HWDGE ring (SP ring carries inputs)
                    nc.scalar.dma_start(
                        y_ap[:, s * C + g0 * NT: s * C + g0 * NT + gc], yg[:]
                    )

    nc.compile()
    return nc


def _get_nc(mode: str):
    if mode not in _BUILT:
        _BUILT[mode] = _build(mode)
    return _BUILT[mode]


# ---------------------------------------------------------------- host wrapper
def _stage_transposed(x_cores: np.ndarray, dtype) -> list[np.ndarray]:
    """x_cores [N_CORES, SEQ_PER_CORE, T] -> per-core padded transposed slabs
    [P, SEQ_PER_CORE*CP] with a leading zero column per sequence."""
    out = []
    for c in range(N_CORES):
        slab = np.zeros((P, SEQ_PER_CORE, CP), dtype=dtype)
        # [8, 3750, 128] -> [128, 8, 3750]
        slab[:, :, 1:] = x_cores[c].reshape(SEQ_PER_CORE, C, P).transpose(2, 0, 1)
        out.append(np.ascontiguousarray(slab.reshape(P, SEQ_PER_CORE * CP)))
    return out


# test-harness hooks: extra kwargs for run_bass_kernel_spmd (e.g. trace=True)
# and the last BassKernelResults (for exec_time_ns / trace paths).
_EXTRA_RUN_KWARGS: dict = {}
_LAST_RESULTS = None


def kernel(waveform) -> np.ndarray:
    global _LAST_RESULTS
    from concourse.bass_utils import run_bass_kernel_spmd

    try:
        x = np.asarray(waveform)
    except Exception:
        # device-resident jax array whose direct transfer path failed
        import jax

        x = np.asarray(jax.device_get(waveform))
    if x.dtype != np.float32:
        x = x.astype(np.float32)
    assert x.shape == (BATCH, T), x.shape

    mode = MODE
    nc = _get_nc(mode)
    wa, wb = _weights()
    x_cores = x.reshape(N_CORES, SEQ_PER_CORE, T)

    if mode == "fp32":
        slabs = _stage_transposed(x_cores, np.float32)
        in_maps = [{"xt": slabs[c], "wa": wa, "wb": wb} for c in range(N_CORES)]
    else:
        bf = ml_dtypes.bfloat16
        xh = x.astype(bf)
        xl = (x - xh.astype(np.float32)).astype(bf)
        wa1 = wa.astype(bf)
        wb1 = wb.astype(bf)
        wa2 = (wa - wa1.astype(np.float32)).astype(bf)
        wb2 = (wb - wb1.astype(np.float32)).astype(bf)
        sh = _stage_transposed(xh.reshape(N_CORES, SEQ_PER_CORE, T), bf)
        sl = _stage_transposed(xl.reshape(N_CORES, SEQ_PER_CORE, T), bf)
        in_maps = [
            {"xh": sh[c], "xl": sl[c], "wa1": wa1, "wb1": wb1,
             "wa2": wa2, "wb2": wb2}
            for c in range(N_CORES)
        ]

    res = run_bass_kernel_spmd(
        nc, in_maps, core_ids=list(range(N_CORES)), **_EXTRA_RUN_KWARGS
    )
    _LAST_RESULTS = res

    y = np.empty((N_CORES, SEQ_PER_CORE, T), dtype=np.float32)
    for c in range(N_CORES):
        yt = res.results[c]["yt"].reshape(P, SEQ_PER_CORE, C)
        # y[s][128c + p] = yt[p, s, c]
        y[c] = yt.transpose(1, 2, 0).reshape(SEQ_PER_CORE, T)
    return y.reshape(BATCH, T)


# revision 20
# speedup vs baseline: 1.0798x; 1.0798x over previous
"""BandPass biquad (torchaudio bandpass_biquad, const_skirt_gain=False) on 8 Trainium2 cores.

Strategy
--------
The biquad is an order-2 IIR with complex poles at radius ~0.691. Its impulse
response decays below 1e-10 after ~64 taps, so in fp32 the filter is *exactly*
(to fp32 precision) a 64-tap causal FIR:  y = conv(x, g),  g = conv([b0,0,b2], h),
h = impulse response of 1/(1 + a1 z^-1 + a2 z^-2).

The convolution maps onto the TensorEngine as banded-Toeplitz matmuls:
  out[q, n] = y[t0 + 128 n + q]
            = sum_p WA[p, q] x[t0 + 128 n + p]  +  sum_p WB[p, q] x[t0 + 128 (n-1) + p]
with WA[p, q] = g[q - p], WB[p, q] = g[q + 128 - p] (zero outside 0 <= . < K).
Two accumulating matmuls per output tile; no sequential recurrence anywhere.

Data is staged host-side into a time-across-partitions layout
x_T[p, c] = x[128 c + p] (one zero pad column per sequence for the t<0 state),
so the device does only full-burst natural DMAs, matmuls, and PSUM->SBUF copies.

Sharding: pure data parallel, 8 sequences per core (batch 64 over 8 cores).

Modes (BANDPASS_MODE env var):
  fp32  - 2 fp32 matmuls / tile.                     rel err ~2e-7
  bf16  - x and g split into bf16 hi+lo parts; 3 group pairs of full-rate bf16
          matmuls (x_hi*g1, x_hi*g2, x_lo*g1) accumulated in fp32 PSUM.
          rel err ~4e-6, ~4x less PE time than fp32.
"""

import math
import os

import ml_dtypes
import numpy as np

# ---------------------------------------------------------------- constants
SR = 48000.0
CENTRAL_FREQ = 4000.0
Q = 0.707

_w0 = 2.0 * math.pi * CENTRAL_FREQ / SR
_alpha = math.sin(_w0) / (2.0 * Q)
_a0 = 1.0 + _alpha
B0 = _alpha / _a0
B2 = -_alpha / _a0
A1 = (-2.0 * math.cos(_w0)) / _a0
A2 = (1.0 - _alpha) / _a0

BATCH, T = 64, 480000
N_CORES = 8
SEQ_PER_CORE = BATCH // N_CORES      # 8
P = 128                              # partitions / conv window
C = T // P                           # 3750 columns per sequence
NT = 375                             # matmul free-dim tile
NTILES = C // NT                     # 10
GROUP = 5                            # tiles per weight-pass group
K = 64                               # FIR taps (|g[63]| ~ 1e-10)
CP = C + 1                           # padded cols per sequence (leading zero col)

MODE = os.environ.get("BANDPASS_MODE", "bf16")


def _taps() -> np.ndarray:
    h = np.zeros(K + 2)
    h[0] = 1.0
    h[1] = -A1
    for n in range(2, K + 2):
        h[n] = -A1 * h[n - 1] - A2 * h[n - 2]
    g = B0 * h
    g[2:] += B2 * h[:-2]
    return g[:K]


def _weights() -> tuple[np.ndarray, np.ndarray]:
    g = _taps()
    p = np.arange(P)[:, None]
    q = np.arange(P)[None, :]
    ka = q - p
    wa = np.where((ka >= 0) & (ka < K), g[np.clip(ka, 0, K - 1)], 0.0)
    kb = q + P - p
    wb = np.where((kb >= 0) & (kb < K), g[np.clip(kb, 0, K - 1)], 0.0)
    return wa.astype(np.float32), wb.astype(np.float32)


# ---------------------------------------------------------------- device kernel
_BUILT = {}


def _build(mode: str):
    """Build the per-core Bass program (same NEFF for all 8 cores)."""
    import concourse.bacc as bacc
    import concourse.mybir as mybir
    import concourse.tile as tile

    nc = bacc.Bacc("TRN2", target_bir_lowering=False, debug=False)

    cp_total = SEQ_PER_CORE * CP
    co_total = SEQ_PER_CORE * C
    f32 = mybir.dt.float32
    bf16 = mybir.dt.bfloat16

    if mode == "fp32":
        x_names = ["xt"]
        w_names = ["wa", "wb"]
        xdt = f32
        # (weight name, x tensor idx, shifted)   shifted=False -> A chunk
        passes = [("wa", 0, False), ("wb", 0, True)]
    else:
        x_names = ["xh", "xl"]
        w_names = ["wa1", "wb1", "wa2", "wb2"]
        xdt = bf16
        passes = [
            ("wa1", 0, False), ("wb1", 0, True),
            ("wa2", 0, False), ("wb2", 0, True),
            ("wa1", 1, False), ("wb1", 1, True),
        ]

    x_aps = [
        nc.dram_tensor(nm, [P, cp_total], xdt, kind="ExternalInput").ap()
        for nm in x_names
    ]
    w_aps = {
        nm: nc.dram_tensor(nm, [P, P], xdt, kind="ExternalInput").ap()
        for nm in w_names
    }
    y_ap = nc.dram_tensor("yt", [P, co_total], f32, kind="ExternalOutput").ap()

    GC = GROUP * NT                  # cols per chunk (1875)
    with tile.TileContext(nc) as tc:
        with (
            tc.tile_pool(name="wpool", bufs=1) as wpool,
            tc.tile_pool(name="xpool", bufs=4) as xpool,
            tc.tile_pool(name="ypool", bufs=4) as ypool,
            tc.tile_pool(name="psum", bufs=7, space="PSUM") as psum_pool,
        ):
            # tiny weight loads first on the SP ring (~128KB total, lands well
            # before the first full x chunk finishes)
            w_tiles = {}
            for nm in w_names:
                wt = wpool.tile([P, P], xdt, tag=nm, name=f"w_{nm}")
                nc.sync.dma_start(wt[:], w_aps[nm][:])
                w_tiles[nm] = wt

            # HAM warm-up: ~3us of dummy matmuls on a zeroed scratch tile so
            # the PE clock-gate is already at 8/8 when the first real data
            # arrives (otherwise the first ~3.4us of real matmuls run at
            # 1.2GHz). Results go to a scratch PSUM tile nothing reads.
            wsc = wpool.tile([P, P], xdt, tag="warm_sc", name="warm_sc")
            nc.gpsimd.memset(wsc[:], 0.0)
            wps = psum_pool.tile([P, P], f32, tag="warm_ps", name="warm_ps",
                                 bufs=1)
            for _ in range(28):
                nc.tensor.matmul(wps[:], wsc[:], wsc[:], start=True, stop=True)

            def groups_for_seq(s):
                # small leading chunks on the first sequence so the PE and the
                # output stream start early; small trailing chunks on the last
                # sequence so the post-last-input tail (matmul+copy+out-DMA)
                # is short. 10 tiles per sequence total.
                if s == 0:
                    return [(0, 1), (1, 2), (3, 2), (5, 5)]
                if s == SEQ_PER_CORE - 1:
                    return [(0, 5), (5, 3), (8, 2)]
                return [(0, GROUP), (GROUP, NTILES - GROUP)]

            for s in range(SEQ_PER_CORE):
                for g0, gn in groups_for_seq(s):
                    gc = gn * NT
                    # x chunk: gc data cols + 1 leading col (zero pad / overlap),
                    # padded-slab cols [s*CP + g0*NT, ... + gc + 1)
                    xs = []
                    for xi, _nm in enumerate(x_names):
                        xc = xpool.tile([P, gc + 1], xdt, tag=f"x{xi}",
                                        name=f"x{xi}_{s}_{g0}")
                        b = s * CP + g0 * NT
                        nc.sync.dma_start(xc[:], x_aps[xi][:, b:b + gc + 1])
                        xs.append(xc)

                    yg = ypool.tile([P, gc], f32, tag="y", name=f"y_{s}_{g0}")
                    ptiles = [
                        psum_pool.tile([P, NT], f32, tag="ps", name=f"ps{t}")
                        for t in range(gn)
                    ]
                    for pi, (wnm, xi, shifted) in enumerate(passes):
                        for t in range(gn):
                            c0 = t * NT + (0 if shifted else 1)
                            nc.tensor.matmul(
                                ptiles[t][:],
                                w_tiles[wnm][:],
                                xs[xi][:, c0:c0 + NT],
                                start=(pi == 0),
                                stop=(pi == len(passes) - 1),
                            )
                    for t in range(gn):
                        nc.vector.tensor_copy(yg[:, t * NT:(t + 1) * NT],
                                              ptiles[t][:])

                    # output chunks on the ACT HWDGE ring (SP ring carries inputs)
                    nc.scalar.dma_start(
                        y_ap[:, s * C + g0 * NT: s * C + g0 * NT + gc], yg[:]
                    )

    nc.compile()
    return nc


def _get_nc(mode: str):
    if mode not in _BUILT:
        _BUILT[mode] = _build(mode)
    return _BUILT[mode]


# ---------------------------------------------------------------- host wrapper
def _stage_transposed(x_cores: np.ndarray, dtype) -> list[np.ndarray]:
    """x_cores [N_CORES, SEQ_PER_CORE, T] -> per-core padded transposed slabs
    [P, SEQ_PER_CORE*CP] with a leading zero column per sequence."""
    out = []
    for c in range(N_CORES):
        slab = np.zeros((P, SEQ_PER_CORE, CP), dtype=dtype)
        # [8, 3750, 128] -> [128, 8, 3750]
        slab[:, :, 1:] = x_cores[c].reshape(SEQ_PER_CORE, C, P).transpose(2, 0, 1)
        out.append(np.ascontiguousarray(slab.reshape(P, SEQ_PER_CORE * CP)))
    return out


# test-harness hooks: extra kwargs for run_bass_kernel_spmd (e.g. trace=True)
# and the last BassKernelResults (for exec_time_ns / trace paths).
_EXTRA_RUN_KWARGS: dict = {}
_LAST_RESULTS = None


def kernel(waveform) -> np.ndarray:
    global _LAST_RESULTS
    from concourse.bass_utils import run_bass_kernel_spmd

    try:
        x = np.asarray(waveform)
    except Exception:
        # device-resident jax array whose direct transfer path failed
        import jax

        x = np.asarray(jax.device_get(waveform))
    if x.dtype != np.float32:
        x = x.astype(np.float32)
    assert x.shape == (BATCH, T), x.shape

    mode = MODE
    nc = _get_nc(mode)
    wa, wb = _weights()
    x_cores = x.reshape(N_CORES, SEQ_PER_CORE, T)

    if mode == "fp32":
        slabs = _stage_transposed(x_cores, np.float32)
        in_maps = [{"xt": slabs[c], "wa": wa, "wb": wb} for c in range(N_CORES)]
    else:
        bf = ml_dtypes.bfloat16
        xh = x.astype(bf)
        xl = (x - xh.astype(np.float32)).astype(bf)
        wa1 = wa.astype(bf)
        wb1 = wb.astype(bf)
        wa2 = (wa - wa1.astype(np.float32)).astype(bf)
        wb2 = (wb - wb1.astype(np.float32)).astype(bf)
        sh = _stage_transposed(xh.reshape(N_CORES, SEQ_PER_CORE, T), bf)
        sl = _stage_transposed(xl.reshape(N_CORES, SEQ_PER_CORE, T), bf)
        in_maps = [
            {"xh": sh[c], "xl": sl[c], "wa1": wa1, "wb1": wb1,
             "wa2": wa2, "wb2": wb2}
            for c in range(N_CORES)
        ]

    res = run_bass_kernel_spmd(
        nc, in_maps, core_ids=list(range(N_CORES)), **_EXTRA_RUN_KWARGS
    )
    _LAST_RESULTS = res

    y = np.empty((N_CORES, SEQ_PER_CORE, T), dtype=np.float32)
    for c in range(N_CORES):
        yt = res.results[c]["yt"].reshape(P, SEQ_PER_CORE, C)
        # y[s][128c + p] = yt[p, s, c]
        y[c] = yt.transpose(1, 2, 0).reshape(SEQ_PER_CORE, T)
    return y.reshape(BATCH, T)


# revision 21
# speedup vs baseline: 1.1595x; 1.0738x over previous
"""BandPass biquad (torchaudio bandpass_biquad, const_skirt_gain=False) on 8 Trainium2 cores.

Strategy
--------
The biquad is an order-2 IIR with complex poles at radius ~0.691. Its impulse
response decays below 1e-10 after ~64 taps, so in fp32 the filter is *exactly*
(to fp32 precision) a 64-tap causal FIR:  y = conv(x, g),  g = conv([b0,0,b2], h),
h = impulse response of 1/(1 + a1 z^-1 + a2 z^-2).

The convolution maps onto the TensorEngine as banded-Toeplitz matmuls:
  out[q, n] = y[t0 + 128 n + q]
            = sum_p WA[p, q] x[t0 + 128 n + p]  +  sum_p WB[p, q] x[t0 + 128 (n-1) + p]
with WA[p, q] = g[q - p], WB[p, q] = g[q + 128 - p] (zero outside 0 <= . < K).
Two accumulating matmuls per output tile; no sequential recurrence anywhere.

Data is staged host-side into a time-across-partitions layout
x_T[p, c] = x[128 c + p] (one zero pad column per sequence for the t<0 state),
so the device does only full-burst natural DMAs, matmuls, and PSUM->SBUF copies.

Sharding: pure data parallel, 8 sequences per core (batch 64 over 8 cores).

Modes (BANDPASS_MODE env var):
  fp32  - 2 fp32 matmuls / tile.                     rel err ~2e-7
  bf16  - x and g split into bf16 hi+lo parts; 3 group pairs of full-rate bf16
          matmuls (x_hi*g1, x_hi*g2, x_lo*g1) accumulated in fp32 PSUM.
          rel err ~4e-6, ~4x less PE time than fp32.
"""

import math
import os

import ml_dtypes
import numpy as np

# ---------------------------------------------------------------- constants
SR = 48000.0
CENTRAL_FREQ = 4000.0
Q = 0.707

_w0 = 2.0 * math.pi * CENTRAL_FREQ / SR
_alpha = math.sin(_w0) / (2.0 * Q)
_a0 = 1.0 + _alpha
B0 = _alpha / _a0
B2 = -_alpha / _a0
A1 = (-2.0 * math.cos(_w0)) / _a0
A2 = (1.0 - _alpha) / _a0

BATCH, T = 64, 480000
N_CORES = 8
SEQ_PER_CORE = BATCH // N_CORES      # 8
P = 128                              # partitions / conv window
C = T // P                           # 3750 columns per sequence
NT = 375                             # matmul free-dim tile
NTILES = C // NT                     # 10
GROUP = 5                            # tiles per weight-pass group
K = 64                               # FIR taps (|g[63]| ~ 1e-10)
CP = C + 1                           # padded cols per sequence (leading zero col)

MODE = os.environ.get("BANDPASS_MODE", "bf16")


def _taps() -> np.ndarray:
    h = np.zeros(K + 2)
    h[0] = 1.0
    h[1] = -A1
    for n in range(2, K + 2):
        h[n] = -A1 * h[n - 1] - A2 * h[n - 2]
    g = B0 * h
    g[2:] += B2 * h[:-2]
    return g[:K]


def _weights() -> tuple[np.ndarray, np.ndarray]:
    g = _taps()
    p = np.arange(P)[:, None]
    q = np.arange(P)[None, :]
    ka = q - p
    wa = np.where((ka >= 0) & (ka < K), g[np.clip(ka, 0, K - 1)], 0.0)
    kb = q + P - p
    wb = np.where((kb >= 0) & (kb < K), g[np.clip(kb, 0, K - 1)], 0.0)
    return wa.astype(np.float32), wb.astype(np.float32)


# ---------------------------------------------------------------- device kernel
_BUILT = {}


def _build(mode: str):
    """Build the per-core Bass program (same NEFF for all 8 cores)."""
    import concourse.bacc as bacc
    import concourse.mybir as mybir
    import concourse.tile as tile

    nc = bacc.Bacc("TRN2", target_bir_lowering=False, debug=False)

    cp_total = SEQ_PER_CORE * CP
    co_total = SEQ_PER_CORE * C
    f32 = mybir.dt.float32
    bf16 = mybir.dt.bfloat16

    if mode == "fp32":
        x_names = ["xt"]
        w_names = ["wa", "wb"]
        xdt = f32
        # (weight name, x tensor idx, shifted)   shifted=False -> A chunk
        passes = [("wa", 0, False), ("wb", 0, True)]
    else:
        x_names = ["xh", "xl"]
        w_names = ["wa1", "wb1", "wa2", "wb2"]
        xdt = bf16
        passes = [
            ("wa1", 0, False), ("wb1", 0, True),
            ("wa2", 0, False), ("wb2", 0, True),
            ("wa1", 1, False), ("wb1", 1, True),
        ]

    x_aps = [
        nc.dram_tensor(nm, [P, cp_total], xdt, kind="ExternalInput").ap()
        for nm in x_names
    ]
    w_aps = {
        nm: nc.dram_tensor(nm, [P, P], xdt, kind="ExternalInput").ap()
        for nm in w_names
    }
    y_ap = nc.dram_tensor("yt", [P, co_total], f32, kind="ExternalOutput").ap()

    GC = GROUP * NT                  # cols per chunk (1875)
    with tile.TileContext(nc) as tc:
        with (
            tc.tile_pool(name="wpool", bufs=1) as wpool,
            tc.tile_pool(name="xpool", bufs=4) as xpool,
            tc.tile_pool(name="ypool", bufs=4) as ypool,
            tc.tile_pool(name="psum", bufs=7, space="PSUM") as psum_pool,
        ):
            # tiny weight loads on the ACT HWDGE ring so they don't delay
            # the first x chunks on the SP ring
            w_tiles = {}
            for nm in w_names:
                wt = wpool.tile([P, P], xdt, tag=nm, name=f"w_{nm}")
                nc.scalar.dma_start(wt[:], w_aps[nm][:])
                w_tiles[nm] = wt

            # HAM warm-up: ~3us of dummy matmuls on a zeroed scratch tile so
            # the PE clock-gate is already at 8/8 when the first real data
            # arrives (otherwise the first ~3.4us of real matmuls run at
            # 1.2GHz). Results go to a scratch PSUM tile nothing reads.
            wsc = wpool.tile([P, P], xdt, tag="warm_sc", name="warm_sc")
            nc.gpsimd.memset(wsc[:], 0.0)
            wps = psum_pool.tile([P, P], f32, tag="warm_ps", name="warm_ps",
                                 bufs=1)
            for _ in range(28):
                nc.tensor.matmul(wps[:], wsc[:], wsc[:], start=True, stop=True)

            def groups_for_seq(s):
                # small leading chunks on the first sequence so the PE and the
                # output stream start early; small trailing chunks on the last
                # sequence so the post-last-input tail (matmul+copy+out-DMA)
                # is short. 10 tiles per sequence total.
                if s == 0:
                    return [(0, 1), (1, 2), (3, 2), (5, 5)]
                if s == SEQ_PER_CORE - 1:
                    return [(0, 5), (5, 3), (8, 2)]
                return [(0, GROUP), (GROUP, NTILES - GROUP)]

            for s in range(SEQ_PER_CORE):
                for g0, gn in groups_for_seq(s):
                    gc = gn * NT
                    # x chunk: gc data cols + 1 leading col (zero pad / overlap),
                    # padded-slab cols [s*CP + g0*NT, ... + gc + 1)
                    xs = []
                    for xi, _nm in enumerate(x_names):
                        xc = xpool.tile([P, gc + 1], xdt, tag=f"x{xi}",
                                        name=f"x{xi}_{s}_{g0}")
                        b = s * CP + g0 * NT
                        nc.sync.dma_start(xc[:], x_aps[xi][:, b:b + gc + 1])
                        xs.append(xc)

                    yg = ypool.tile([P, gc], f32, tag="y", name=f"y_{s}_{g0}")
                    ptiles = [
                        psum_pool.tile([P, NT], f32, tag="ps", name=f"ps{t}")
                        for t in range(gn)
                    ]
                    for pi, (wnm, xi, shifted) in enumerate(passes):
                        for t in range(gn):
                            c0 = t * NT + (0 if shifted else 1)
                            nc.tensor.matmul(
                                ptiles[t][:],
                                w_tiles[wnm][:],
                                xs[xi][:, c0:c0 + NT],
                                start=(pi == 0),
                                stop=(pi == len(passes) - 1),
                            )
                    for t in range(gn):
                        nc.vector.tensor_copy(yg[:, t * NT:(t + 1) * NT],
                                              ptiles[t][:])

                    # output chunks on the ACT HWDGE ring (SP ring carries inputs)
                    nc.scalar.dma_start(
                        y_ap[:, s * C + g0 * NT: s * C + g0 * NT + gc], yg[:]
                    )

    nc.compile()
    return nc


def _get_nc(mode: str):
    if mode not in _BUILT:
        _BUILT[mode] = _build(mode)
    return _BUILT[mode]


# ---------------------------------------------------------------- host wrapper
def _stage_transposed(x_cores: np.ndarray, dtype) -> list[np.ndarray]:
    """x_cores [N_CORES, SEQ_PER_CORE, T] -> per-core padded transposed slabs
    [P, SEQ_PER_CORE*CP] with a leading zero column per sequence."""
    out = []
    for c in range(N_CORES):
        slab = np.zeros((P, SEQ_PER_CORE, CP), dtype=dtype)
        # [8, 3750, 128] -> [128, 8, 3750]
        slab[:, :, 1:] = x_cores[c].reshape(SEQ_PER_CORE, C, P).transpose(2, 0, 1)
        out.append(np.ascontiguousarray(slab.reshape(P, SEQ_PER_CORE * CP)))
    return out


# test-harness hooks: extra kwargs for run_bass_kernel_spmd (e.g. trace=True)
# and the last BassKernelResults (for exec_time_ns / trace paths).
_EXTRA_RUN_KWARGS: dict = {}
_LAST_RESULTS = None


def kernel(waveform) -> np.ndarray:
    global _LAST_RESULTS
    from concourse.bass_utils import run_bass_kernel_spmd

    try:
        x = np.asarray(waveform)
    except Exception:
        # device-resident jax array whose direct transfer path failed
        import jax

        x = np.asarray(jax.device_get(waveform))
    if x.dtype != np.float32:
        x = x.astype(np.float32)
    assert x.shape == (BATCH, T), x.shape

    mode = MODE
    nc = _get_nc(mode)
    wa, wb = _weights()
    x_cores = x.reshape(N_CORES, SEQ_PER_CORE, T)

    if mode == "fp32":
        slabs = _stage_transposed(x_cores, np.float32)
        in_maps = [{"xt": slabs[c], "wa": wa, "wb": wb} for c in range(N_CORES)]
    else:
        bf = ml_dtypes.bfloat16
        xh = x.astype(bf)
        xl = (x - xh.astype(np.float32)).astype(bf)
        wa1 = wa.astype(bf)
        wb1 = wb.astype(bf)
        wa2 = (wa - wa1.astype(np.float32)).astype(bf)
        wb2 = (wb - wb1.astype(np.float32)).astype(bf)
        sh = _stage_transposed(xh.reshape(N_CORES, SEQ_PER_CORE, T), bf)
        sl = _stage_transposed(xl.reshape(N_CORES, SEQ_PER_CORE, T), bf)
        in_maps = [
            {"xh": sh[c], "xl": sl[c], "wa1": wa1, "wb1": wb1,
             "wa2": wa2, "wb2": wb2}
            for c in range(N_CORES)
        ]

    res = run_bass_kernel_spmd(
        nc, in_maps, core_ids=list(range(N_CORES)), **_EXTRA_RUN_KWARGS
    )
    _LAST_RESULTS = res

    y = np.empty((N_CORES, SEQ_PER_CORE, T), dtype=np.float32)
    for c in range(N_CORES):
        yt = res.results[c]["yt"].reshape(P, SEQ_PER_CORE, C)
        # y[s][128c + p] = yt[p, s, c]
        y[c] = yt.transpose(1, 2, 0).reshape(SEQ_PER_CORE, T)
    return y.reshape(BATCH, T)
